# revision 35
# baseline (speedup 1.0000x reference)
"""Trainium2 Bass kernel for AfmoeAttention (sliding-window GQA attention with
QK-norm, RoPE, sigmoid gating).

Sharding: 8 cores = 2 batches x 4 head-groups. Core c handles batch c//4 and
q-heads [4*(c%4), 4*(c%4)+4) with kv-head c%4. Each core computes a partial
o_proj output [S, HID] (its 4 heads' contribution); host sums the 4 partials
per batch (in float64).

Layout strategy (no on-chip fp32 transposes needed anywhere):
  - x passed transposed: xT [HID, S]; weights passed transposed [HID, oc].
  - q/k projections produce qT/kT [head_dim, t] (head dim on partitions); v is
    produced as vT then PE-transposed to token-major [t, D] for PV.
  - q/k head channels permuted host-side to de-interleave RoPE pairs: within a
    head block, [even dims(64); odd dims(64)] -> RoPE is partition-aligned.
  - scores computed transposed: sT[j, i] (keys on partitions): softmax sum over
    keys is a PE ones-matmul; per-query scaling via rank-1 broadcast matmuls.
  - all transcendentals use one ACT table set (ln/exp): rsqrt = exp(-.5 ln),
    1/x = exp(-ln x), sigmoid folded into exp(-(ln den + ln(1+e^-g))).
  - matmuls in float32r (~1.5e-4 rel err, 4x faster than fp32 on TRN2).

NOTE: q_norm_w/k_norm_w are folded into wq/wk host-side, which is exact when
they are constant per RoPE pair (the spec fills them with ones).
"""
import sys

if "/opt/trn_rl_repo" not in sys.path:
    sys.path.insert(0, "/opt/trn_rl_repo")

import numpy as np

import concourse.bass as bass
import concourse.tile as tile
from concourse import bacc, mybir
from concourse.bass_utils import run_bass_kernel_spmd

# Pin every activation to one table set so the table-load pass never thrashes
# (square/ln/exp/copy/identity are all present in natural_log_exp_and_others).
import concourse.hw_specs as _hw_specs
_ORIG_GAT = _hw_specs.get_activation_tables

def _gat_pinned(arch):
    tabs = _ORIG_GAT(arch)
    keep = "natural_log_exp_and_others"
    return {n: (f if n == keep else set()) for n, f in tabs.items()}

bacc.get_activation_tables = _gat_pinned

F32 = mybir.dt.float32
F32R = mybir.dt.float32r
AF = mybir.ActivationFunctionType

B, S, HID = 2, 2048, 2048
H, HK, D = 16, 4, 128
WINDOW = 1024
EPS = 1e-6
P = 128
NCORES = 8
HPC = 4          # q heads per core
NSRC = HPC + 1   # norm-stat sources: 4 q heads + k
TQ = 512         # token/query tile
NT = S // TQ
KT = HID // P    # 16 contraction tiles
SCALE = D ** -0.5
NEG = -30000.0


def _jtile_plan(t0):
    """(j0, c0, n, mask) per key tile for query block [t0, t0+TQ).

    c0/n: computed query-column range [c0, c0+n). mask: None|("c",m0)|("w",m0)
    = [128,128] mask add at cols [m0, m0+128). First entry covers all TQ cols.
    """
    plan = []
    for j0 in range(max(0, t0 - WINDOW), t0 + TQ - P + 1, P):
        d = t0 - j0
        if d <= 0:
            plan.append((j0, -d, TQ + d, ("c", -d)))
        elif d >= WINDOW - TQ + P:      # window-partial
            n = P + WINDOW - d
            plan.append((j0, 0, n, ("w", n - P)))
        else:
            plan.append((j0, 0, TQ, None))
    plan.sort(key=lambda e: 0 if e[1] == 0 and e[2] == TQ and e[3] is not None else 1)
    assert plan[0][1] == 0 and plan[0][2] == TQ
    return plan


PHASE_MARKS = []


def build_nc():
    PHASE_MARKS.clear()
    nc = bacc.Bacc("TRN2", target_bir_lowering=False, debug=False,
                   num_devices=NCORES)
    eps_t = nc.alloc_sbuf_tensor("const-eps", [128, 1], F32)
    nc.gpsimd.memset(eps_t.ap(), EPS)
    nc.const_aps.aps[(F32, EPS)] = eps_t.ap()
    nc.all_engine_barrier()

    d_in = lambda name, shape, dt=F32R: nc.dram_tensor(name, shape, dt, kind="ExternalInput")
    xT_d = d_in("xT", [NT, P, KT * TQ])   # host-prearranged per t-tile
    wqT_d = d_in("wqT", [HID, HPC * D])
    wkT_d = d_in("wkT", [HID, D])
    wvT_d = d_in("wvT", [HID, D])
    wgT_d = d_in("wgT", [HID, HPC * D])
    woT_d = d_in("woT", [HPC * D, HID])
    cc_d = d_in("cc", [P, S], F32)       # [cos; cos]
    ss_d = d_in("ss", [P, S], F32)       # [-sin; sin]
    mc_d = d_in("mc", [P, P], F32)       # causal mask add (0 / NEG)
    mw_d = d_in("mw", [P, P], F32)       # window mask add
    sel_d = d_in("sel", [P, NSRC * NSRC])  # block h: col h = w^-2(perm), else 0
    ones_d = d_in("ones", [P, 1])
    bsel_d = d_in("bsel", [NSRC, P * NSRC])  # block h: row h = ones
    dsel_d = d_in("dsel", [P, HPC * HPC])    # block h: col h = ones
    onesr_d = d_in("onesr", [1, P])
    ident_d = d_in("ident", [P, P])
    out_d = nc.dram_tensor("out", [S, HID], F32, kind="ExternalOutput")

    with tile.TileContext(nc) as tc:
        with tc.tile_pool(name="wpool", bufs=1) as wpool, \
             tc.tile_pool(name="const", bufs=1) as cpool, \
             tc.tile_pool(name="resid", bufs=1) as rpool, \
             tc.tile_pool(name="xp", bufs=1) as xp, \
             tc.tile_pool(name="qg", bufs=1) as qg, \
             tc.tile_pool(name="work", bufs=2) as work, \
             tc.tile_pool(name="work2", bufs=1) as work2, \
             tc.tile_pool(name="wos", bufs=2) as wos, \
             tc.tile_pool(name="probs", bufs=4) as prp, \
             tc.tile_pool(name="ccss", bufs=1) as ccss, \
             tc.tile_pool(name="ostage", bufs=3) as ostage, \
             tc.tile_pool(name="ps", bufs=1, space="PSUM") as ps:

            # ---- resident weights + constants (one DMA each) ----
            wq = wpool.tile([P, KT, HPC * D], F32R)
            wk = wpool.tile([P, KT, D], F32R)
            wv = wpool.tile([P, KT, D], F32R)
            wg = wpool.tile([P, KT, HPC * D], F32R)
            for kc in range(0, KT, 4):
                nc.scalar.dma_start(
                    wq[:, kc:kc + 4],
                    wqT_d.rearrange("(k p) o -> p k o", p=P)[:, kc:kc + 4])
                nc.scalar.dma_start(
                    wg[:, kc:kc + 4],
                    wgT_d.rearrange("(k p) o -> p k o", p=P)[:, kc:kc + 4])
            nc.scalar.dma_start(wk[:], wkT_d.rearrange("(k p) o -> p k o", p=P))
            nc.scalar.dma_start(wv[:], wvT_d.rearrange("(k p) o -> p k o", p=P))
            mc = cpool.tile([P, P], F32); nc.scalar.dma_start(mc[:], mc_d[:])
            mw = cpool.tile([P, P], F32); nc.scalar.dma_start(mw[:], mw_d[:])
            sel = cpool.tile([P, NSRC * NSRC], F32R); nc.scalar.dma_start(sel[:], sel_d[:])
            ones = cpool.tile([P, 1], F32R); nc.scalar.dma_start(ones[:], ones_d[:])
            bsel = cpool.tile([NSRC, P * NSRC], F32R); nc.scalar.dma_start(bsel[:], bsel_d[:])
            dsel = cpool.tile([P, HPC * HPC], F32R); nc.scalar.dma_start(dsel[:], dsel_d[:])
            onesr = cpool.tile([1, P], F32R); nc.scalar.dma_start(onesr[:], onesr_d[:])
            ident = cpool.tile([P, P], F32R); nc.scalar.dma_start(ident[:], ident_d[:])

            # ---- resident activations ----
            kTr = rpool.tile([P, S], F32R)
            vtok = [rpool.tile([P, D], F32R, tag=f"v{j}", name=f"v{j}") for j in range(S // P)]

            for ti in range(NT):
                t0 = ti * TQ
                xbA = xp.tile([P, KT * TQ // 2], F32R, tag="xbA")
                xbB = xp.tile([P, KT * TQ // 2], F32R, tag="xbB")
                nc.sync.dma_start(xbA[:], xT_d[ti, :, :KT * TQ // 2])
                nc.sync.dma_start(xbB[:], xT_d[ti, :, KT * TQ // 2:])
                xs = [(xbA if k < KT // 2 else xbB)
                      [:, (k % (KT // 2)) * TQ:((k % (KT // 2)) + 1) * TQ]
                      for k in range(KT)]

                # ---- pass A: q(4), k, vT projections ----
                sc_passA = nc.enter_named_scope("passA", False); PHASE_MARKS.append((nc.next_id(), "passA"))
                ps_q = [ps.tile([P, TQ], F32, tag=f"q{h}", name=f"psq{h}") for h in range(HPC)]
                ps_k = ps.tile([P, TQ], F32, tag="k")
                ps_vT = ps.tile([P, TQ], F32, tag="vT")
                for k in range(KT):
                    st, sp = (k == 0), (k == KT - 1)
                    for h in range(HPC):
                        nc.tensor.matmul(ps_q[h][:], wq[:, k, h * D:(h + 1) * D],
                                         xs[k], start=st, stop=sp)
                    nc.tensor.matmul(ps_k[:], wk[:, k], xs[k], start=st, stop=sp)
                    nc.tensor.matmul(ps_vT[:], wv[:, k], xs[k], start=st, stop=sp)

                nc.leave_named_scope("passA", sc_passA[0], False)
                sc_vtr = nc.enter_named_scope("vtr", False); PHASE_MARKS.append((nc.next_id(), "vtr"))
                vT_sb = work2.tile([P, TQ], F32R, tag="vTsb")
                nc.vector.tensor_copy(vT_sb[:], ps_vT[:])
                for st4 in range(TQ // P):
                    ps_t = ps.tile([P, P], F32R, tag="vT", name="ps_t")
                    nc.tensor.transpose(ps_t[:], vT_sb[:, st4 * P:(st4 + 1) * P], ident[:])
                    nc.vector.tensor_copy(vtok[t0 // P + st4][:], ps_t[:])

                nc.leave_named_scope("vtr", sc_vtr[0], False)
                sc_stat = nc.enter_named_scope("stats", False); PHASE_MARKS.append((nc.next_id(), "stats"))
                ps_stat = ps.tile([NSRC, TQ], F32, tag="stat")
                for h in range(NSRC):
                    sq = work.tile([P, TQ], F32R, tag="scr")
                    nc.scalar.square(sq[:], (ps_k if h == HPC else ps_q[h])[:])
                    nc.tensor.matmul(ps_stat[:], sel[:, h * NSRC:(h + 1) * NSRC],
                                     sq[:], start=(h == 0), stop=(h == NSRC - 1))
                lnm = work.tile([P, TQ], F32, tag="scr", name="lnm")
                nc.scalar.activation(lnm[0:NSRC, :], ps_stat[:], AF.Ln,
                                     bias=EPS, scale=1.0 / D)
                sinv = work2.tile([NSRC, TQ], F32R, tag="sinv")
                nc.scalar.activation(sinv[:], lnm[0:NSRC, :], AF.Exp, scale=-0.5)

                nc.leave_named_scope("stats", sc_stat[0], False)
                sc_rope = nc.enter_named_scope("rope", False); PHASE_MARKS.append((nc.next_id(), "rope"))
                cct = ccss.tile([P, TQ], F32, tag="cc")
                sst = ccss.tile([P, TQ], F32, tag="ss")
                nc.scalar.dma_start(cct[:], cc_d[:, t0:t0 + TQ])
                nc.scalar.dma_start(sst[:], ss_d[:, t0:t0 + TQ])

                # ---- rope + norm-scale for q heads and k ----
                qT = [qg.tile([P, TQ], F32R, tag=f"qT{h}", name=f"qT{h}") for h in range(HPC)]
                for h in range(NSRC):
                    src = ps_k if h == HPC else ps_q[h]
                    tmp = work.tile([P, TQ], F32, tag="scr", name="rtmp")
                    nc.vector.tensor_copy(tmp[0:64, :], src[64:128, :])
                    nc.vector.tensor_copy(tmp[64:128, :], src[0:64, :])
                    rr = work.tile([P, TQ], F32, tag="rr")
                    nc.vector.tensor_mul(rr[:], src[:], cct[:])
                    nc.vector.tensor_mul(tmp[:], tmp[:], sst[:])
                    nc.vector.tensor_add(rr[:], rr[:], tmp[:])
                    ps_b = ps.tile([P, TQ], F32, tag="bcast", name="ps_b")
                    nc.tensor.matmul(ps_b[:], bsel[:, h * P:(h + 1) * P], sinv[:],
                                     start=True, stop=True)
                    dst = kTr[:, t0:t0 + TQ] if h == HPC else qT[h][:]
                    nc.vector.tensor_mul(dst, rr[:], ps_b[:])

                nc.leave_named_scope("rope", sc_rope[0], False)
                sc_gate = nc.enter_named_scope("gate", False); PHASE_MARKS.append((nc.next_id(), "gate"))
                gl = [qg.tile([P, TQ], F32, tag=f"g{h}", name=f"g{h}") for h in range(HPC)]
                for h in range(HPC):
                    ps_g = ps.tile([P, TQ], F32, tag=f"q{h}", name=f"ps_g{h}")
                    for k in range(KT):
                        nc.tensor.matmul(ps_g[:], wg[:, k, h * D:(h + 1) * D],
                                         xs[k], start=(k == 0), stop=(k == KT - 1))
                    eg = work.tile([P, TQ], F32, tag="scr", name="eg")
                    nc.scalar.activation(eg[:], ps_g[:], AF.Exp, scale=-1.0)
                    nc.vector.tensor_scalar_add(eg[:], eg[:], 1.0)
                    nc.scalar.activation(gl[h][:], eg[:], AF.Ln)
                nc.leave_named_scope("gate", sc_gate[0], False)
                sc_att = nc.enter_named_scope("attn", False); PHASE_MARKS.append((nc.next_id(), "attn"))
                plan = _jtile_plan(t0)
                oT = [qg.tile([P, TQ], F32R, tag=f"oT{h}", name=f"oT{h}") for h in range(HPC)]
                ps_den = ps.tile([HPC, TQ], F32, tag="stat", name="ps_den")
                ps_os = []
                for h in range(HPC):
                    ps_o = ps.tile([P, TQ], F32, tag=["q3", "bcast", "k", "vT"][h],
                                   name="ps_o")
                    ps_os.append(ps_o)
                    first = True
                    for si, (j0, c0, n, msk) in enumerate(plan):
                        last = si == len(plan) - 1
                        ps_s = ps.tile([P, TQ], F32, tag=f"q{si % 3}", name="ps_s")
                        nc.tensor.matmul(ps_s[:, c0:c0 + n], kTr[:, j0:j0 + P],
                                         qT[h][:, c0:c0 + n], start=True, stop=True)
                        if msk is not None:
                            kind, m0 = msk
                            mt = mc if kind == "c" else mw
                            nc.vector.tensor_add(ps_s[:, m0:m0 + P],
                                                 ps_s[:, m0:m0 + P], mt[:])
                        pr = prp.tile([P, TQ], F32R, tag="pr")
                        nc.scalar.activation(pr[:, c0:c0 + n], ps_s[:, c0:c0 + n],
                                             AF.Exp, scale=SCALE)
                        nc.tensor.matmul(ps_o[:, c0:c0 + n], vtok[j0 // P][:],
                                         pr[:, c0:c0 + n], start=first, stop=last)
                        nc.tensor.matmul(ps_den[:, c0:c0 + n],
                                         dsel[:, h * HPC:(h + 1) * HPC],
                                         pr[:, c0:c0 + n],
                                         start=(first and h == 0),
                                         stop=(last and h == HPC - 1),
                                         skip_group_check=True)
                        first = False
                lden = work2.tile([HPC, TQ], F32R, tag="lden")
                nc.scalar.activation(lden[:], ps_den[:], AF.Ln)
                for h in range(HPC):
                    ps_b2 = ps.tile([P, TQ], F32, tag="q0" if h % 2 == 0 else "q1",
                                    name="ps_b2")
                    nc.tensor.matmul(ps_b2[:], bsel[0:HPC, h * P:(h + 1) * P],
                                     lden[:], start=True, stop=True)
                    ex = work.tile([P, TQ], F32, tag="scr", name="ex")
                    nc.vector.tensor_add(ex[:], ps_b2[:], gl[h][:])
                    mult = work.tile([P, TQ], F32, tag="scr", name="mult")
                    nc.scalar.activation(mult[:], ex[:], AF.Exp, scale=-1.0)
                    nc.vector.tensor_mul(oT[h][:], ps_os[h][:], mult[:])

                nc.leave_named_scope("attn", sc_att[0], False)
                sc_op = nc.enter_named_scope("oproj", False); PHASE_MARKS.append((nc.next_id(), "oproj"))
                for n4 in range(HID // 512):
                    wot = wos.tile([P, HPC, 512], F32R, tag="wot")
                    nc.sync.dma_start(
                        wot[:], woT_d.rearrange("(k p) o -> p k o", p=P)
                        [:, :, n4 * 512:(n4 + 1) * 512])
                    for st4 in range(TQ // P):
                        ps_out = ps.tile([P, 512], F32,
                                         tag=["k", "vT", "q3", "bcast"][st4],
                                         name="ps_out")
                        for k in range(HPC):
                            nc.tensor.matmul(ps_out[:], oT[k][:, st4 * P:(st4 + 1) * P],
                                             wot[:, k], start=(k == 0), stop=(k == HPC - 1))
                        ob = ostage.tile([P, 512], F32, tag="ob")
                        if st4 % 2 == 0:
                            nc.vector.tensor_copy(ob[:], ps_out[:])
                        else:
                            nc.scalar.copy(ob[:], ps_out[:])
                        nc.sync.dma_start(
                            out_d[t0 + st4 * P:t0 + (st4 + 1) * P,
                                  n4 * 512:(n4 + 1) * 512], ob[:])
                nc.leave_named_scope("oproj", sc_op[0], False)
    nc.finalize()
    return nc


_PERM = np.concatenate([np.arange(0, D, 2), np.arange(1, D, 2)])


def host_inputs(x, freqs_cis, wq, wk, wv, wo, wgate, q_norm_w, k_norm_w):
    x = np.asarray(x, np.float32)
    freqs_cis = np.asarray(freqs_cis, np.float32)
    wq = np.asarray(wq, np.float32); wk = np.asarray(wk, np.float32)
    wv = np.asarray(wv, np.float32); wo = np.asarray(wo, np.float32)
    wgate = np.asarray(wgate, np.float32)
    q_norm_w = np.asarray(q_norm_w, np.float32)
    k_norm_w = np.asarray(k_norm_w, np.float32)

    cos = np.ascontiguousarray(freqs_cis[:, :, 0].T)   # [64, S]
    sin = np.ascontiguousarray(freqs_cis[:, :, 1].T)
    cc = np.concatenate([cos, cos], 0)
    ss = np.concatenate([-sin, sin], 0)
    ii = np.arange(P)
    mcm = np.where(ii[None, :] >= ii[:, None], 0.0, NEG).astype(np.float32)
    mwm = np.where(ii[None, :] <= ii[:, None], 0.0, NEG).astype(np.float32)
    ident = np.eye(P, dtype=np.float32)
    onesc = np.ones((P, 1), np.float32)
    onesr = np.ones((1, P), np.float32)
    bsel = np.zeros((NSRC, P * NSRC), np.float32)
    for h in range(NSRC):
        bsel[h, h * P:(h + 1) * P] = 1.0
    dsel = np.zeros((P, HPC * HPC), np.float32)
    for h in range(HPC):
        dsel[:, h * HPC + h] = 1.0

    qw = np.where(q_norm_w == 0, 1.0, q_norm_w)
    kw = np.where(k_norm_w == 0, 1.0, k_norm_w)
    sel = np.zeros((P, NSRC * NSRC), np.float32)
    for h in range(HPC):
        sel[:, h * NSRC + h] = (qw ** -2.0)[_PERM]
    sel[:, HPC * NSRC + HPC] = (kw ** -2.0)[_PERM]

    xTs = []
    for b in range(B):
        xt = x[b].T.reshape(KT, P, NT, TQ)        # [k, p, ti, t']
        xTs.append(np.ascontiguousarray(xt.transpose(2, 1, 0, 3)
                                        .reshape(NT, P, KT * TQ)))

    in_maps = []
    for c in range(NCORES):
        b, hg = divmod(c, HPC)
        rows = slice(hg * HPC * D, (hg + 1) * HPC * D)
        wq_c = wq[rows] * np.tile(q_norm_w, HPC)[:, None]
        wq_p = wq_c.reshape(HPC, D, HID)[:, _PERM, :].reshape(HPC * D, HID)
        wk_c = (wk[hg * D:(hg + 1) * D] * k_norm_w[:, None])[_PERM, :]
        in_maps.append({
            "xT": xTs[b],
            "wqT": np.ascontiguousarray(wq_p.T),
            "wkT": np.ascontiguousarray(wk_c.T),
            "wvT": np.ascontiguousarray(wv[hg * D:(hg + 1) * D].T),
            "wgT": np.ascontiguousarray(wgate[rows].T),
            "woT": np.ascontiguousarray(wo[:, rows].T),
            "cc": cc, "ss": ss, "mc": mcm, "mw": mwm,
            "sel": sel, "ones": onesc, "onesr": onesr, "ident": ident,
            "bsel": bsel, "dsel": dsel,
        })
    return in_maps


_NC = None


def kernel(**inputs):
    global _NC
    in_maps = host_inputs(**inputs)
    last_err = None
    for attempt in range(3):
        try:
            if _NC is None:
                _NC = build_nc()
            res = run_bass_kernel_spmd(_NC, in_maps, list(range(NCORES)))
            break
        except Exception as e:  # transient tunneled-device errors: retry
            last_err = e
            _NC = None
            import time
            time.sleep(5 * (attempt + 1))
    else:
        raise last_err
    out = np.zeros((B, S, HID), np.float64)
    for c in range(NCORES):
        out[c // HPC] += res.results[c]["out"].astype(np.float64)
    return out.astype(np.float32)
